# revision 15
# baseline (speedup 1.0000x reference)
"""AUGRU kernel for Trainium2 (Bass/Tile), 8-core data-parallel.

Problem: T=100 steps of an attention-gated GRU over B=8192, D_IN=UNITS=128.
    u = sigmoid(x Wu + bu + h Uu)
    r = sigmoid(x Wr + br + h Ur)
    c = tanh(x Wc + bc + r * (h Uc))
    h = (1 - att*u) * h + att*u * c
Output: final h [B, UNITS] fp32.

Design notes:
- Pure data parallel: batch sharded 8 ways (1024 per core), weights replicated.
- Feature-major layout on chip: h kept as hT [UNITS=128 partitions, B free];
  the transposes of x and state are done host-side.
- fp16 everywhere on-chip (absmax err ~2.6e-3 vs fp32 reference); PSUM fp32.
- Two independent 512-column batch chains per core run in anti-phase so each
  engine alternates between them (the per-step dependency chain is ~5us).
- r and c gates use persistent PSUM accumulators (zr, zch) updated with
  m = h(t) - h(t-1) instead of h itself:
      zr  += m@Ur + x(t)@Wr - x(t-1)@Wr ;  zch += m@Uc
  This takes the final h-update (hn = hp + m) off the critical path: the next
  step's r-chain starts from m, not from hn.
- r*(h Uc) is accumulated into the candidate PSUM bank via an identity matmul.
- GPSIMD does no elementwise work (it shares an SBUF port with the DVE and
  stalls it), so all elementwise ops run on the DVE.
- att broadcast across partitions via DMA (partition-stride-0 read from HBM).
- Per step, emission is two-phase: both chains' gate matmuls + sigmoids +
  products first, then both chains' identity-accumulate + tanh + combine, so
  the in-order PE never has chain B's matmuls queued behind chain A's
  data-dependent identity matmul.

Measured (8 cores, full problem): ~517us HW exec (chip fast clock state;
~620us when the chip DVFS-throttles to 5/6 clocks). Output absmax error
vs fp32: ~2.4e-3, norm rel err ~1.1e-3. Engine occupancy at 517us: DVE 83%
(the binding engine: 10 fp16 TTs/step of which 2 read PSUM at 1x),
PE 76% (16 matmuls/step), ACT 71% (6 activations/step).
"""

import numpy as np

T, B, D, U = 100, 8192, 128, 128
NCORES = 8
BL = B // NCORES  # 1024 batch elements per core

_compiled = None  # (nc, biases_zero) cache


def _build(biases_zero: bool):
    import concourse.bacc as bacc
    import concourse.mybir as mybir
    import concourse.tile as tile
    from concourse import masks

    f16 = mybir.dt.float16
    f32 = mybir.dt.float32
    AF = mybir.ActivationFunctionType

    nc = bacc.Bacc(
        "TRN2",
        target_bir_lowering=False,
        debug=False,
        enable_asserts=False,
        num_devices=NCORES,
    )

    xT = nc.dram_tensor("xT", (T, D, BL), f16, kind="ExternalInput").ap()
    h0T = nc.dram_tensor("h0T", (U, BL), f16, kind="ExternalInput").ap()
    attb = nc.dram_tensor("attb", (T, BL), f16, kind="ExternalInput").ap()
    # weight slots: 0=Wu, 1=Wr, 2=Wc, 3=-Wr
    wx = nc.dram_tensor("wx", (4, D, U), f16, kind="ExternalInput").ap()
    wh = nc.dram_tensor("wh", (3, U, U), f16, kind="ExternalInput").ap()
    bias = nc.dram_tensor("bias", (3, U), f32, kind="ExternalInput").ap()
    out = nc.dram_tensor("hT_out", (U, BL), f16, kind="ExternalOutput").ap()

    NCH = 2
    CW = BL // NCH  # 512

    with tile.TileContext(nc) as tc:
        with (
            tc.tile_pool(name="singles", bufs=1) as singles,
            tc.tile_pool(name="xp", bufs=4) as xp,
            tc.tile_pool(name="ap_", bufs=4) as apool,
            tc.tile_pool(name="urp", bufs=3) as urp,
            tc.tile_pool(name="ew", bufs=3) as ew,
            tc.tile_pool(name="mp", bufs=3) as mp,
            tc.tile_pool(name="psum", bufs=1, space="PSUM") as psum,
        ):
            WU, WR, WC, WRN = (slice(U * i, U * (i + 1)) for i in range(4))

            wx_sb = singles.tile([D, 4 * U], f16, tag="wx")
            wh_sb = singles.tile([U, 3 * U], f16, tag="wh")
            for i in range(4):
                nc.sync.dma_start(wx_sb[:, U * i : U * (i + 1)], wx[i])
            for i in range(3):
                nc.sync.dma_start(wh_sb[:, U * i : U * (i + 1)], wh[i])

            ident = singles.tile([128, 128], f16, tag="ident")
            masks.make_identity(nc, ident[:])

            bias_sb = None
            if not biases_zero:
                bias_sb = singles.tile([U, 3], f32, tag="bias")
                for i in range(3):
                    nc.sync.dma_start(bias_sb[:, i : i + 1], bias[i][:, None])

            h_sb = [
                [
                    singles.tile([U, CW], f16, tag=f"h{k}_{p}", name=f"h_{k}_{p}")
                    for p in range(2)
                ]
                for k in range(NCH)
            ]
            for k in range(NCH):
                nc.sync.dma_start(h_sb[k][0][:], h0T[:, CW * k : CW * (k + 1)])

            # persistent PSUM accumulators (never freed; 4 of the 8 banks)
            zr = [psum.tile([128, CW], f32, tag=f"zr{k}", name=f"zr{k}")
                  for k in range(NCH)]
            zch = [psum.tile([128, CW], f32, tag=f"zch{k}", name=f"zch{k}")
                   for k in range(NCH)]

            m_prev = [None, None]
            x_prev = None

            for t in range(T):
                xt = xp.tile([D, BL], f16, tag="xt", name=f"xt_{t}")
                nc.sync.dma_start(xt[:], xT[t])
                ab = apool.tile([128, BL], f16, tag="ab", name=f"ab_{t}")
                nc.sync.dma_start(ab[:], attb[t][None, :].broadcast_to((128, BL)))

                ph2 = {}
                for k in range(NCH):
                    cs = slice(CW * k, CW * (k + 1))
                    xs = xt[:, cs]
                    hp = h_sb[k][t % 2][:]
                    hn = h_sb[k][(t + 1) % 2][:]

                    # --- r/ch accumulator updates ---
                    if t == 0:
                        nc.tensor.matmul(zr[k][:], wh_sb[:, WR], hp,
                                         start=True, stop=False,
                                         skip_group_check=True)
                        nc.tensor.matmul(zr[k][:], wx_sb[:, WR], xs,
                                         start=False, stop=False,
                                         skip_group_check=True)
                        nc.tensor.matmul(zch[k][:], wh_sb[:, WC], hp,
                                         start=True, stop=False,
                                         skip_group_check=True)
                    else:
                        nc.tensor.matmul(zr[k][:], wh_sb[:, WR], m_prev[k][:],
                                         start=False, stop=False,
                                         skip_group_check=True)
                        nc.tensor.matmul(zr[k][:], wx_sb[:, WR], xs,
                                         start=False, stop=False,
                                         skip_group_check=True)
                        nc.tensor.matmul(zr[k][:], wx_sb[:, WRN], x_prev[:, cs],
                                         start=False, stop=False,
                                         skip_group_check=True)
                        nc.tensor.matmul(zch[k][:], wh_sb[:, WC], m_prev[k][:],
                                         start=False, stop=False,
                                         skip_group_check=True)

                    # candidate x-part BEFORE the pu matmuls: pc is on the
                    # tanh critical path and depends only on x, while the
                    # hn-driven pu Uu-matmul is the latest-ready matmul and
                    # would otherwise stall the in-order PE ahead of pc
                    pu = psum.tile([128, CW], f32, tag=f"pu{k}", name=f"pu{k}_{t}")
                    pc = psum.tile([128, CW], f32, tag=f"pc{k}", name=f"pc{k}_{t}")
                    nc.tensor.matmul(pc[:], wx_sb[:, WC], xs,
                                     start=True, stop=False, skip_group_check=True)

                    rb = 0.0 if biases_zero else bias_sb[:, 1:2]
                    ub = 0.0 if biases_zero else bias_sb[:, 0:1]
                    cb = 0.0 if biases_zero else bias_sb[:, 2:3]

                    # r sigmoid (path-critical) from the accumulator
                    r = urp.tile([128, CW], f16, tag=f"r{k}", name=f"r{k}_{t}")
                    nc.scalar.activation(r[:], zr[k][:], AF.Sigmoid, bias=rb)

                    # u gate (its h-side matmul waits on last step's hn)
                    nc.tensor.matmul(pu[:], wx_sb[:, WU], xs,
                                     start=True, stop=False)
                    nc.tensor.matmul(pu[:], wh_sb[:, WU], hp,
                                     start=False, stop=True)
                    u = urp.tile([128, CW], f16, tag=f"u{k}", name=f"u{k}_{t}")
                    nc.scalar.activation(u[:], pu[:], AF.Sigmoid, bias=ub)

                    # t1 = r * zch
                    t1 = ew.tile([128, CW], f16, tag=f"t1{k}", name=f"t1{k}_{t}")
                    nc.vector.tensor_mul(t1[:], r[:], zch[k][:])

                    # g = att * u
                    g = ew.tile([128, CW], f16, tag=f"g{k}", name=f"g{k}_{t}")
                    nc.vector.tensor_mul(g[:], u[:], ab[:, cs])
                    ph2[k] = (pc, t1, g, hp, hn, cb)

                # phase 2: identity-accumulate + tanh + combine, after both
                # chains' gate matmuls are already queued on the PE
                for k in range(NCH):
                    pc, t1, g, hp, hn, cb = ph2[k]
                    nc.tensor.matmul(pc[:], ident[:], t1[:],
                                     start=False, stop=True, skip_group_check=True)

                    # c = tanh(pc [+ bc])
                    c = ew.tile([128, CW], f16, tag=f"c{k}", name=f"c{k}_{t}")
                    nc.scalar.activation(c[:], pc[:], AF.Tanh, bias=cb)

                    # d = c - h, m = g*d (m feeds next step's accumulators),
                    # hn = h + m (off the critical path now)
                    dd = ew.tile([128, CW], f16, tag=f"d{k}", name=f"d{k}_{t}")
                    nc.vector.tensor_sub(dd[:], c[:], hp)
                    m = mp.tile([128, CW], f16, tag=f"m{k}", name=f"m{k}_{t}")
                    nc.vector.tensor_mul(m[:], g[:], dd[:])
                    nc.vector.tensor_add(hn, hp, m[:])
                    m_prev[k] = m
                x_prev = xt

            for k in range(NCH):
                nc.sync.dma_start(out[:, CW * k : CW * (k + 1)],
                                  h_sb[k][T % 2][:])

    nc.compile()
    return nc


def _prep_inputs(inputs):
    """Host-side layout transforms (feature-major + fp16) -> per-core in_maps."""
    x = np.asarray(inputs["inputs"], dtype=np.float32)
    state = np.asarray(inputs["state"], dtype=np.float32)
    att = np.asarray(inputs["att_score"], dtype=np.float32)

    xT = np.ascontiguousarray(x.transpose(0, 2, 1)).astype(np.float16)       # [T, D, B]
    h0T = np.ascontiguousarray(state.T).astype(np.float16)                   # [U, B]
    attb = np.ascontiguousarray(att[:, :, 0]).astype(np.float16)             # [T, B]
    Wr16 = np.asarray(inputs["Wr"]).astype(np.float16)
    wx = np.stack([np.asarray(inputs["Wu"]).astype(np.float16), Wr16,
                   np.asarray(inputs["Wc"]).astype(np.float16), -Wr16])      # [4, D, U]
    wh = np.stack([inputs["Uu"], inputs["Ur"], inputs["Uc"]]).astype(np.float16)
    bias = np.stack([inputs["bu"], inputs["br"], inputs["bc"]]).astype(np.float32)

    in_maps = []
    for k in range(NCORES):
        s = slice(k * BL, (k + 1) * BL)
        in_maps.append({
            "xT": np.ascontiguousarray(xT[:, :, s]),
            "h0T": np.ascontiguousarray(h0T[:, s]),
            "attb": np.ascontiguousarray(attb[:, s]),
            "wx": wx,
            "wh": wh,
            "bias": bias,
        })
    return in_maps


def kernel(**inputs):
    global _compiled
    biases_zero = not (np.any(inputs["bu"]) or np.any(inputs["br"])
                       or np.any(inputs["bc"]))

    if _compiled is None or _compiled[1] != biases_zero:
        _compiled = (_build(biases_zero), biases_zero)
    nc = _compiled[0]

    in_maps = _prep_inputs(inputs)

    from concourse.bass_utils import run_bass_kernel_spmd

    res = run_bass_kernel_spmd(nc, in_maps, core_ids=list(range(NCORES)))

    out = np.empty((B, U), dtype=np.float32)
    for k in range(NCORES):
        out[k * BL : (k + 1) * BL, :] = res.results[k]["hT_out"].T.astype(np.float32)
    return out


# revision 21
# speedup vs baseline: 1.0837x; 1.0837x over previous
"""AUGRU kernel for Trainium2 (Bass/Tile), 8-core data-parallel.

Problem: T=100 steps of an attention-gated GRU over B=8192, D_IN=UNITS=128.
    u = sigmoid(x Wu + bu + h Uu)
    r = sigmoid(x Wr + br + h Ur)
    c = tanh(x Wc + bc + r * (h Uc))
    h = (1 - att*u) * h + att*u * c
Output: final h [B, UNITS] fp32.

Design notes:
- Pure data parallel: batch sharded 8 ways (1024 per core), weights replicated.
- Feature-major layout on chip: h kept as hT [UNITS=128 partitions, B free];
  the transposes of x and state are done host-side.
- fp16 everywhere on-chip (absmax err ~2.6e-3 vs fp32 reference); PSUM fp32.
- Two independent 512-column batch chains per core run in anti-phase so each
  engine alternates between them (the per-step dependency chain is ~5us).
- r and c gates use persistent PSUM accumulators (zr, zch) updated with
  m = h(t) - h(t-1) instead of h itself:
      zr  += m@Ur + x(t)@Wr - x(t-1)@Wr ;  zch += m@Uc
  This takes the final h-update (hn = hp + m) off the critical path: the next
  step's r-chain starts from m, not from hn.
- r*(h Uc) is accumulated into the candidate PSUM bank via an identity matmul.
- GPSIMD does no elementwise work (it shares an SBUF port with the DVE and
  stalls it), so all elementwise ops run on the DVE.
- att broadcast across partitions via DMA (partition-stride-0 read from HBM).
- Per step, emission is two-phase: both chains' gate matmuls + sigmoids +
  products first, then both chains' identity-accumulate + tanh + combine, so
  the in-order PE never has chain B's matmuls queued behind chain A's
  data-dependent identity matmul.

Measured (8 cores, full problem): ~517us HW exec (chip fast clock state;
~620us when the chip DVFS-throttles to 5/6 clocks). Output absmax error
vs fp32: ~2.4e-3, norm rel err ~1.1e-3. Engine occupancy at 517us: DVE 83%
(the binding engine: 10 fp16 TTs/step of which 2 read PSUM at 1x),
PE 76% (16 matmuls/step), ACT 71% (6 activations/step).
"""

import numpy as np

T, B, D, U = 100, 8192, 128, 128
NCORES = 8
BL = B // NCORES  # 1024 batch elements per core

_compiled = None  # (nc, biases_zero) cache


def _build(biases_zero: bool):
    import concourse.bacc as bacc
    import concourse.mybir as mybir
    import concourse.tile as tile
    from concourse import masks

    f16 = mybir.dt.float16
    f32 = mybir.dt.float32
    AF = mybir.ActivationFunctionType

    nc = bacc.Bacc(
        "TRN2",
        target_bir_lowering=False,
        debug=False,
        enable_asserts=False,
        num_devices=NCORES,
    )

    xT = nc.dram_tensor("xT", (T, D, BL), f16, kind="ExternalInput").ap()
    h0T = nc.dram_tensor("h0T", (U, BL), f16, kind="ExternalInput").ap()
    attb = nc.dram_tensor("attb", (T, BL), f16, kind="ExternalInput").ap()
    # weight slots: 0=Wu, 1=Wr, 2=Wc, 3=-Wr, 4=-Wu
    wx = nc.dram_tensor("wx", (5, D, U), f16, kind="ExternalInput").ap()
    wh = nc.dram_tensor("wh", (3, U, U), f16, kind="ExternalInput").ap()
    bias = nc.dram_tensor("bias", (3, U), f32, kind="ExternalInput").ap()
    out = nc.dram_tensor("hT_out", (U, BL), f16, kind="ExternalOutput").ap()

    NCH = 2
    CW = BL // NCH  # 512

    with tile.TileContext(nc) as tc:
        with (
            tc.tile_pool(name="singles", bufs=1) as singles,
            tc.tile_pool(name="xp", bufs=4) as xp,
            tc.tile_pool(name="ap_", bufs=4) as apool,
            tc.tile_pool(name="urp", bufs=3) as urp,
            tc.tile_pool(name="ew", bufs=3) as ew,
            tc.tile_pool(name="mp", bufs=3) as mp,
            tc.tile_pool(name="psum", bufs=1, space="PSUM") as psum,
        ):
            WU, WR, WC, WRN, WUN = (slice(U * i, U * (i + 1)) for i in range(5))

            wx_sb = singles.tile([D, 5 * U], f16, tag="wx")
            wh_sb = singles.tile([U, 3 * U], f16, tag="wh")
            for i in range(5):
                nc.sync.dma_start(wx_sb[:, U * i : U * (i + 1)], wx[i])
            for i in range(3):
                nc.sync.dma_start(wh_sb[:, U * i : U * (i + 1)], wh[i])

            ident = singles.tile([128, 128], f16, tag="ident")
            masks.make_identity(nc, ident[:])

            bias_sb = None
            if not biases_zero:
                bias_sb = singles.tile([U, 3], f32, tag="bias")
                for i in range(3):
                    nc.sync.dma_start(bias_sb[:, i : i + 1], bias[i][:, None])

            h_sb = [
                [
                    singles.tile([U, CW], f16, tag=f"h{k}_{p}", name=f"h_{k}_{p}")
                    for p in range(2)
                ]
                for k in range(NCH)
            ]
            for k in range(NCH):
                nc.sync.dma_start(h_sb[k][0][:], h0T[:, CW * k : CW * (k + 1)])

            # persistent PSUM accumulators (never freed; 6 of the 8 banks)
            zr = [psum.tile([128, CW], f32, tag=f"zr{k}", name=f"zr{k}")
                  for k in range(NCH)]
            zch = [psum.tile([128, CW], f32, tag=f"zch{k}", name=f"zch{k}")
                   for k in range(NCH)]
            zu = [psum.tile([128, CW], f32, tag=f"zu{k}", name=f"zu{k}")
                  for k in range(NCH)]

            m_prev = [None, None]
            x_prev = None

            for t in range(T):
                xt = xp.tile([D, BL], f16, tag="xt", name=f"xt_{t}")
                nc.sync.dma_start(xt[:], xT[t])
                ab = apool.tile([128, BL], f16, tag="ab", name=f"ab_{t}")
                nc.sync.dma_start(ab[:], attb[t][None, :].broadcast_to((128, BL)))

                ph2 = {}
                for k in range(NCH):
                    cs = slice(CW * k, CW * (k + 1))
                    xs = xt[:, cs]
                    hp = h_sb[k][t % 2][:]
                    hn = h_sb[k][(t + 1) % 2][:]

                    # --- r/ch/u accumulator updates ---
                    if t == 0:
                        nc.tensor.matmul(zr[k][:], wh_sb[:, WR], hp,
                                         start=True, stop=False,
                                         skip_group_check=True)
                        nc.tensor.matmul(zr[k][:], wx_sb[:, WR], xs,
                                         start=False, stop=False,
                                         skip_group_check=True)
                        nc.tensor.matmul(zch[k][:], wh_sb[:, WC], hp,
                                         start=True, stop=False,
                                         skip_group_check=True)
                        nc.tensor.matmul(zu[k][:], wh_sb[:, WU], hp,
                                         start=True, stop=False,
                                         skip_group_check=True)
                        nc.tensor.matmul(zu[k][:], wx_sb[:, WU], xs,
                                         start=False, stop=False,
                                         skip_group_check=True)
                    else:
                        nc.tensor.matmul(zr[k][:], wh_sb[:, WR], m_prev[k][:],
                                         start=False, stop=False,
                                         skip_group_check=True)
                        nc.tensor.matmul(zr[k][:], wx_sb[:, WR], xs,
                                         start=False, stop=False,
                                         skip_group_check=True)
                        nc.tensor.matmul(zr[k][:], wx_sb[:, WRN], x_prev[:, cs],
                                         start=False, stop=False,
                                         skip_group_check=True)
                        nc.tensor.matmul(zch[k][:], wh_sb[:, WC], m_prev[k][:],
                                         start=False, stop=False,
                                         skip_group_check=True)
                        nc.tensor.matmul(zu[k][:], wh_sb[:, WU], m_prev[k][:],
                                         start=False, stop=False,
                                         skip_group_check=True)
                        nc.tensor.matmul(zu[k][:], wx_sb[:, WU], xs,
                                         start=False, stop=False,
                                         skip_group_check=True)
                        nc.tensor.matmul(zu[k][:], wx_sb[:, WUN], x_prev[:, cs],
                                         start=False, stop=False,
                                         skip_group_check=True)

                    # candidate x-part (pc is on the tanh critical path)
                    pc = psum.tile([128, CW], f32, tag=f"pc{k}", name=f"pc{k}_{t}")
                    nc.tensor.matmul(pc[:], wx_sb[:, WC], xs,
                                     start=True, stop=False, skip_group_check=True)

                    rb = 0.0 if biases_zero else bias_sb[:, 1:2]
                    ub = 0.0 if biases_zero else bias_sb[:, 0:1]
                    cb = 0.0 if biases_zero else bias_sb[:, 2:3]

                    # r sigmoid (path-critical) from the accumulator
                    r = urp.tile([128, CW], f16, tag=f"r{k}", name=f"r{k}_{t}")
                    nc.scalar.activation(r[:], zr[k][:], AF.Sigmoid, bias=rb)
                    u = urp.tile([128, CW], f16, tag=f"u{k}", name=f"u{k}_{t}")
                    nc.scalar.activation(u[:], zu[k][:], AF.Sigmoid, bias=ub)

                    # t1 = r * zch
                    t1 = ew.tile([128, CW], f16, tag=f"t1{k}", name=f"t1{k}_{t}")
                    nc.vector.tensor_mul(t1[:], r[:], zch[k][:])

                    # g = att * u
                    g = ew.tile([128, CW], f16, tag=f"g{k}", name=f"g{k}_{t}")
                    nc.vector.tensor_mul(g[:], u[:], ab[:, cs])
                    ph2[k] = (pc, t1, g, hp, hn, cb)

                # phase 2: identity-accumulate + tanh + combine, after both
                # chains' gate matmuls are already queued on the PE
                for k in range(NCH):
                    pc, t1, g, hp, hn, cb = ph2[k]
                    nc.tensor.matmul(pc[:], ident[:], t1[:],
                                     start=False, stop=True, skip_group_check=True)

                    # c = tanh(pc [+ bc])
                    c = ew.tile([128, CW], f16, tag=f"c{k}", name=f"c{k}_{t}")
                    nc.scalar.activation(c[:], pc[:], AF.Tanh, bias=cb)

                    # d = c - h, m = g*d (m feeds next step's accumulators),
                    # hn = h + m (off the critical path now)
                    dd = ew.tile([128, CW], f16, tag=f"d{k}", name=f"d{k}_{t}")
                    nc.vector.tensor_sub(dd[:], c[:], hp)
                    m = mp.tile([128, CW], f16, tag=f"m{k}", name=f"m{k}_{t}")
                    nc.vector.tensor_mul(m[:], g[:], dd[:])
                    nc.vector.tensor_add(hn, hp, m[:])
                    m_prev[k] = m
                x_prev = xt

            for k in range(NCH):
                nc.sync.dma_start(out[:, CW * k : CW * (k + 1)],
                                  h_sb[k][T % 2][:])

    nc.compile()
    return nc


def _prep_inputs(inputs):
    """Host-side layout transforms (feature-major + fp16) -> per-core in_maps."""
    x = np.asarray(inputs["inputs"], dtype=np.float32)
    state = np.asarray(inputs["state"], dtype=np.float32)
    att = np.asarray(inputs["att_score"], dtype=np.float32)

    xT = np.ascontiguousarray(x.transpose(0, 2, 1)).astype(np.float16)       # [T, D, B]
    h0T = np.ascontiguousarray(state.T).astype(np.float16)                   # [U, B]
    attb = np.ascontiguousarray(att[:, :, 0]).astype(np.float16)             # [T, B]
    Wr16 = np.asarray(inputs["Wr"]).astype(np.float16)
    Wu16 = np.asarray(inputs["Wu"]).astype(np.float16)
    wx = np.stack([Wu16, Wr16, np.asarray(inputs["Wc"]).astype(np.float16),
                   -Wr16, -Wu16])                                            # [5, D, U]
    wh = np.stack([inputs["Uu"], inputs["Ur"], inputs["Uc"]]).astype(np.float16)
    bias = np.stack([inputs["bu"], inputs["br"], inputs["bc"]]).astype(np.float32)

    in_maps = []
    for k in range(NCORES):
        s = slice(k * BL, (k + 1) * BL)
        in_maps.append({
            "xT": np.ascontiguousarray(xT[:, :, s]),
            "h0T": np.ascontiguousarray(h0T[:, s]),
            "attb": np.ascontiguousarray(attb[:, s]),
            "wx": wx,
            "wh": wh,
            "bias": bias,
        })
    return in_maps


def kernel(**inputs):
    global _compiled
    biases_zero = not (np.any(inputs["bu"]) or np.any(inputs["br"])
                       or np.any(inputs["bc"]))

    if _compiled is None or _compiled[1] != biases_zero:
        _compiled = (_build(biases_zero), biases_zero)
    nc = _compiled[0]

    in_maps = _prep_inputs(inputs)

    from concourse.bass_utils import run_bass_kernel_spmd

    res = run_bass_kernel_spmd(nc, in_maps, core_ids=list(range(NCORES)))

    out = np.empty((B, U), dtype=np.float32)
    for k in range(NCORES):
        out[k * BL : (k + 1) * BL, :] = res.results[k]["hT_out"].T.astype(np.float32)
    return out


# revision 23
# speedup vs baseline: 1.1984x; 1.1059x over previous
"""AUGRU kernel for Trainium2 (Bass/Tile), 8-core data-parallel.

Problem: T=100 steps of an attention-gated GRU over B=8192, D_IN=UNITS=128.
    u = sigmoid(x Wu + bu + h Uu)
    r = sigmoid(x Wr + br + h Ur)
    c = tanh(x Wc + bc + r * (h Uc))
    h = (1 - att*u) * h + att*u * c
Output: final h [B, UNITS] fp32.

Design notes:
- Pure data parallel: batch sharded 8 ways (1024 per core), weights replicated.
- Feature-major layout on chip: h kept as hT [UNITS=128 partitions, B free];
  the transposes of x and state are done host-side.
- fp16 everywhere on-chip (absmax err ~2.6e-3 vs fp32 reference); PSUM fp32.
- Two independent 512-column batch chains per core run in anti-phase so each
  engine alternates between them (the per-step dependency chain is ~5us).
- r and c gates use persistent PSUM accumulators (zr, zch) updated with
  m = h(t) - h(t-1) instead of h itself:
      zr  += m@Ur + x(t)@Wr - x(t-1)@Wr ;  zch += m@Uc
  This takes the final h-update (hn = hp + m) off the critical path: the next
  step's r-chain starts from m, not from hn.
- r*(h Uc) is accumulated into the candidate PSUM bank via an identity matmul.
- GPSIMD does no elementwise work (it shares an SBUF port with the DVE and
  stalls it), so all elementwise ops run on the DVE.
- att broadcast across partitions via DMA (partition-stride-0 read from HBM).
"""

import numpy as np

T, B, D, U = 100, 8192, 128, 128
NCORES = 8
BL = B // NCORES  # 1024 batch elements per core

_compiled = None  # (nc, biases_zero) cache


def _build(biases_zero: bool):
    import concourse.bacc as bacc
    import concourse.mybir as mybir
    import concourse.tile as tile
    from concourse import masks

    f16 = mybir.dt.float16
    f32 = mybir.dt.float32
    AF = mybir.ActivationFunctionType

    nc = bacc.Bacc(
        "TRN2",
        target_bir_lowering=False,
        debug=False,
        enable_asserts=False,
        num_devices=NCORES,
    )

    xT = nc.dram_tensor("xT", (T, D, BL), f16, kind="ExternalInput").ap()
    h0T = nc.dram_tensor("h0T", (U, BL), f16, kind="ExternalInput").ap()
    attb = nc.dram_tensor("attb", (T, BL), f16, kind="ExternalInput").ap()
    # weight slots: 0=Wu, 1=Wr, 2=Wc, 3=-Wr
    wx = nc.dram_tensor("wx", (4, D, U), f16, kind="ExternalInput").ap()
    wh = nc.dram_tensor("wh", (3, U, U), f16, kind="ExternalInput").ap()
    bias = nc.dram_tensor("bias", (3, U), f32, kind="ExternalInput").ap()
    out = nc.dram_tensor("hT_out", (U, BL), f16, kind="ExternalOutput").ap()

    NCH = 2
    CW = BL // NCH  # 512

    with tile.TileContext(nc) as tc:
        with (
            tc.tile_pool(name="singles", bufs=1) as singles,
            tc.tile_pool(name="xp", bufs=4) as xp,
            tc.tile_pool(name="ap_", bufs=4) as apool,
            tc.tile_pool(name="urp", bufs=3) as urp,
            tc.tile_pool(name="ew", bufs=3) as ew,
            tc.tile_pool(name="mp", bufs=3) as mp,
            tc.tile_pool(name="psum", bufs=1, space="PSUM") as psum,
        ):
            WU, WR, WC, WRN = (slice(U * i, U * (i + 1)) for i in range(4))

            wx_sb = singles.tile([D, 4 * U], f16, tag="wx")
            wh_sb = singles.tile([U, 3 * U], f16, tag="wh")
            for i in range(4):
                nc.sync.dma_start(wx_sb[:, U * i : U * (i + 1)], wx[i])
            for i in range(3):
                nc.sync.dma_start(wh_sb[:, U * i : U * (i + 1)], wh[i])

            ident = singles.tile([128, 128], f16, tag="ident")
            masks.make_identity(nc, ident[:])

            bias_sb = None
            if not biases_zero:
                bias_sb = singles.tile([U, 3], f32, tag="bias")
                for i in range(3):
                    nc.sync.dma_start(bias_sb[:, i : i + 1], bias[i][:, None])

            h_sb = [
                [
                    singles.tile([U, CW], f16, tag=f"h{k}_{p}", name=f"h_{k}_{p}")
                    for p in range(2)
                ]
                for k in range(NCH)
            ]
            for k in range(NCH):
                nc.sync.dma_start(h_sb[k][0][:], h0T[:, CW * k : CW * (k + 1)])

            # persistent PSUM accumulators (never freed; 4 of the 8 banks)
            zr = [psum.tile([128, CW], f32, tag=f"zr{k}", name=f"zr{k}")
                  for k in range(NCH)]
            zch = [psum.tile([128, CW], f32, tag=f"zch{k}", name=f"zch{k}")
                   for k in range(NCH)]

            m_prev = [None, None]
            xts, abs_ = {}, {}
            ph2 = {}

            rb = 0.0 if biases_zero else bias_sb[:, 1:2]
            ub = 0.0 if biases_zero else bias_sb[:, 0:1]
            cb = 0.0 if biases_zero else bias_sb[:, 2:3]

            def dmas(t):
                xts[t] = xp.tile([D, BL], f16, tag="xt", name=f"xt_{t}")
                nc.sync.dma_start(xts[t][:], xT[t])
                abs_[t] = apool.tile([128, BL], f16, tag="ab", name=f"ab_{t}")
                nc.sync.dma_start(abs_[t][:],
                                  attb[t][None, :].broadcast_to((128, BL)))

            def phase1(k, t):
                cs = slice(CW * k, CW * (k + 1))
                xs = xts[t][:, cs]
                hp = h_sb[k][t % 2][:]
                # r/ch accumulator updates
                if t == 0:
                    nc.tensor.matmul(zr[k][:], wh_sb[:, WR], hp,
                                     start=True, stop=False,
                                     skip_group_check=True)
                    nc.tensor.matmul(zr[k][:], wx_sb[:, WR], xs,
                                     start=False, stop=False,
                                     skip_group_check=True)
                    nc.tensor.matmul(zch[k][:], wh_sb[:, WC], hp,
                                     start=True, stop=False,
                                     skip_group_check=True)
                else:
                    nc.tensor.matmul(zr[k][:], wh_sb[:, WR], m_prev[k][:],
                                     start=False, stop=False,
                                     skip_group_check=True)
                    nc.tensor.matmul(zr[k][:], wx_sb[:, WR], xs,
                                     start=False, stop=False,
                                     skip_group_check=True)
                    nc.tensor.matmul(zr[k][:], wx_sb[:, WRN],
                                     xts[t - 1][:, cs],
                                     start=False, stop=False,
                                     skip_group_check=True)
                    nc.tensor.matmul(zch[k][:], wh_sb[:, WC], m_prev[k][:],
                                     start=False, stop=False,
                                     skip_group_check=True)

                # u gate and candidate x-part (regular PSUM tiles)
                pu = psum.tile([128, CW], f32, tag=f"pu{k}", name=f"pu{k}_{t}")
                pc = psum.tile([128, CW], f32, tag=f"pc{k}", name=f"pc{k}_{t}")
                nc.tensor.matmul(pu[:], wx_sb[:, WU], xs,
                                 start=True, stop=False)
                nc.tensor.matmul(pu[:], wh_sb[:, WU], hp,
                                 start=False, stop=True)
                nc.tensor.matmul(pc[:], wx_sb[:, WC], xs,
                                 start=True, stop=False, skip_group_check=True)

                # r sigmoid (path-critical) from the accumulator
                r = urp.tile([128, CW], f16, tag=f"r{k}", name=f"r{k}_{t}")
                nc.scalar.activation(r[:], zr[k][:], AF.Sigmoid, bias=rb)
                u = urp.tile([128, CW], f16, tag=f"u{k}", name=f"u{k}_{t}")
                nc.scalar.activation(u[:], pu[:], AF.Sigmoid, bias=ub)

                # t1 = r * zch
                t1 = ew.tile([128, CW], f16, tag=f"t1{k}", name=f"t1{k}_{t}")
                nc.vector.tensor_mul(t1[:], r[:], zch[k][:])

                # g = att * u
                g = ew.tile([128, CW], f16, tag=f"g{k}", name=f"g{k}_{t}")
                nc.vector.tensor_mul(g[:], u[:], abs_[t][:, cs])
                ph2[k] = (pc, t1, g)

            def phase2(k, t):
                pc, t1, g = ph2[k]
                hp = h_sb[k][t % 2][:]
                hn = h_sb[k][(t + 1) % 2][:]
                nc.tensor.matmul(pc[:], ident[:], t1[:],
                                 start=False, stop=True, skip_group_check=True)
                # c = tanh(pc [+ bc])
                c = ew.tile([128, CW], f16, tag=f"c{k}", name=f"c{k}_{t}")
                nc.scalar.activation(c[:], pc[:], AF.Tanh, bias=cb)
                # d = c - h, m = g*d (m feeds next step's accumulators),
                # hn = h + m (off the critical path)
                dd = ew.tile([128, CW], f16, tag=f"d{k}", name=f"d{k}_{t}")
                nc.vector.tensor_sub(dd[:], c[:], hp)
                m = mp.tile([128, CW], f16, tag=f"m{k}", name=f"m{k}_{t}")
                nc.vector.tensor_mul(m[:], g[:], dd[:])
                nc.vector.tensor_add(hn, hp, m[:])
                m_prev[k] = m

            # explicit half-step software pipeline: chain B runs half a step
            # behind chain A, so every engine FIFO receives ops in temporal
            # order (no head-of-line blocking of A's tanh behind B's sigmoids)
            dmas(0)
            phase1(0, 0)
            for t in range(T):
                phase2(0, t)
                phase1(1, t)
                if t + 1 < T:
                    dmas(t + 1)
                    phase1(0, t + 1)
                phase2(1, t)
                xts.pop(t - 1, None)
                abs_.pop(t - 1, None)

            for k in range(NCH):
                nc.sync.dma_start(out[:, CW * k : CW * (k + 1)],
                                  h_sb[k][T % 2][:])

    nc.compile()
    return nc


def _prep_inputs(inputs):
    """Host-side layout transforms (feature-major + fp16) -> per-core in_maps."""
    x = np.asarray(inputs["inputs"], dtype=np.float32)
    state = np.asarray(inputs["state"], dtype=np.float32)
    att = np.asarray(inputs["att_score"], dtype=np.float32)

    xT = np.ascontiguousarray(x.transpose(0, 2, 1)).astype(np.float16)       # [T, D, B]
    h0T = np.ascontiguousarray(state.T).astype(np.float16)                   # [U, B]
    attb = np.ascontiguousarray(att[:, :, 0]).astype(np.float16)             # [T, B]
    Wr16 = np.asarray(inputs["Wr"]).astype(np.float16)
    wx = np.stack([np.asarray(inputs["Wu"]).astype(np.float16), Wr16,
                   np.asarray(inputs["Wc"]).astype(np.float16), -Wr16])      # [4, D, U]
    wh = np.stack([inputs["Uu"], inputs["Ur"], inputs["Uc"]]).astype(np.float16)
    bias = np.stack([inputs["bu"], inputs["br"], inputs["bc"]]).astype(np.float32)

    in_maps = []
    for k in range(NCORES):
        s = slice(k * BL, (k + 1) * BL)
        in_maps.append({
            "xT": np.ascontiguousarray(xT[:, :, s]),
            "h0T": np.ascontiguousarray(h0T[:, s]),
            "attb": np.ascontiguousarray(attb[:, s]),
            "wx": wx,
            "wh": wh,
            "bias": bias,
        })
    return in_maps


def kernel(**inputs):
    global _compiled
    biases_zero = not (np.any(inputs["bu"]) or np.any(inputs["br"])
                       or np.any(inputs["bc"]))

    if _compiled is None or _compiled[1] != biases_zero:
        _compiled = (_build(biases_zero), biases_zero)
    nc = _compiled[0]

    in_maps = _prep_inputs(inputs)

    from concourse.bass_utils import run_bass_kernel_spmd

    res = run_bass_kernel_spmd(nc, in_maps, core_ids=list(range(NCORES)))

    out = np.empty((B, U), dtype=np.float32)
    for k in range(NCORES):
        out[k * BL : (k + 1) * BL, :] = res.results[k]["hT_out"].T.astype(np.float32)
    return out
